# revision 5
# baseline (speedup 1.0000x reference)
"""Multi-head causal attention on 8 Trainium2 NeuronCores.

Problem: X [2, 2048, 1024] f32, W_q/W_k/W_v [1024, 1024], W_o [1024, 1024],
b_o [1024]; 16 heads, head_dim 64, causal softmax attention + out projection.

Sharding: 2 (batch) x 4 (head-blocks of 4 heads) = 8 cores. Each core
computes q/k/v for its 4 heads on its batch, causal attention, and a partial
output projection ctx @ W_o[rows]. Host sums the 4 partials per batch and
adds b_o. No cross-core collectives.

Per-core kernel (all matmuls fp32r = FP22 multiplies, fp32 accumulate):
  - transpose X -> XT [1024, 2048] via PE identity matmuls
  - qT/kT per head-pair stacked [128, 2048] (A: partitions 0-63, B: 64-127)
  - v tiles [128 keys, 192]: [vA | ones | pad | ones | pad | vB] so a sliding
    128-col lhsT window yields ctxA+denomA (rows 0-64) from one matmul and
    denomB+ctxB (rows 32, 64-127) from another -- denominators ride along as
    ones-columns, and the two heads' ctx land on disjoint partition halves
    for lane-aligned stacking into the out-proj lhsT.
  - scores per (pair, q-chunk 512, k-block 128): two row-packed matmuls
    (K=64 at array rows 0-63 / 64-127) -> one [128, 1024] PSUM tile; causal
    masks added on diagonal blocks; one exp over both banks (scale=1/8).
  - softmax normalization: reciprocal of denom rows + ones-row broadcast
    matmuls + lane-aligned DVE muls into ctxn [128, 2048] per pair.
  - out-proj: out[t, j] += ctxn_p[:, t-block].T @ Wo_p[:, j-chunk].
"""
import sys

sys.path.insert(0, "/opt/trn_rl_repo")

import numpy as np

NEG = -1.0e9
B, NTOK, DIN = 2, 2048, 1024
NH, HD = 16, 64
HPC = 4            # heads per core
CLOC = HPC * HD    # 256 local channels
NCORES = 8
NTB = NTOK // 128  # 16 token blocks
NQC = NTOK // 512  # 4 q-chunks
NCB = DIN // 128   # 8 contraction blocks

_CACHE = {}


def _build():
    from concourse import bacc, mybir, tile

    F32 = mybir.dt.float32
    F32R = mybir.dt.float32r
    EXP = mybir.ActivationFunctionType.Exp
    P = 128

    nc = bacc.Bacc(None)
    Xl = nc.declare_dram_parameter("Xl", [NTOK, DIN], F32, isOutput=False)
    Wq = nc.declare_dram_parameter("Wq", [DIN, CLOC], F32, isOutput=False)
    Wk = nc.declare_dram_parameter("Wk", [DIN, CLOC], F32, isOutput=False)
    Wv = nc.declare_dram_parameter("Wv", [DIN, CLOC], F32, isOutput=False)
    Wo = nc.declare_dram_parameter("Wo", [CLOC, DIN], F32, isOutput=False)
    masks = nc.declare_dram_parameter("masks", [4, P, 512], F32, isOutput=False)
    onesrow = nc.declare_dram_parameter("onesrow", [1, P], F32, isOutput=False)
    ident = nc.declare_dram_parameter("ident", [P, P], F32, isOutput=False)
    out = nc.declare_dram_parameter("out", [NTOK, DIN], F32, isOutput=True)

    with tile.TileContext(nc) as tc:
        with (
            tc.tile_pool(name="const", bufs=1) as constp,
            tc.tile_pool(name="qkT", bufs=1) as qkTp,
            tc.tile_pool(name="vt", bufs=1) as vtp,
            tc.tile_pool(name="ctxn", bufs=1) as ctxnp,
            tc.tile_pool(name="wo", bufs=1) as wop,
        ):
            ident_sb = constp.tile([P, P], F32, tag="ident")
            nc.sync.dma_start(ident_sb[:], ident[:])
            mask_sb = constp.tile([P, 4, 512], F32, tag="masks")
            nc.sync.dma_start(mask_sb[:], masks[:].rearrange("o p f -> p o f"))
            ones_sb = constp.tile([P, P], F32R, tag="ones")
            nc.sync.dma_start(
                ones_sb[:], onesrow[:].to_broadcast((P, P)).bitcast(F32R)
            )

            # static result tiles
            qT = [qkTp.tile([P, NTOK], F32R, tag=f"qT{p}", name=f"qT{p}") for p in range(2)]
            kT = [qkTp.tile([P, NTOK], F32R, tag=f"kT{p}", name=f"kT{p}") for p in range(2)]
            vt = [
                [vtp.tile([P, 192], F32R, tag=f"vt{p}_{tb}", name=f"vt{p}_{tb}") for tb in range(NTB)]
                for p in range(2)
            ]
            ctxn = [ctxnp.tile([P, NTOK], F32R, tag=f"ctxn{p}", name=f"ctxn{p}") for p in range(2)]
            wo_sb = [wop.tile([P, DIN], F32R, tag=f"wo{p}", name=f"wo{p}") for p in range(2)]
            for p in range(2):
                nc.sync.dma_start(
                    wo_sb[p][:], Wo[p * P:(p + 1) * P, :].bitcast(F32R)
                )

            # ---------------- phase A: X load + transpose + QKV ----------------
            with (
                tc.tile_pool(name="xt", bufs=1) as xtp,
                tc.tile_pool(name="wqkv", bufs=1) as wp,
                tc.tile_pool(name="xin", bufs=3) as xinp,
                tc.tile_pool(name="psA", bufs=1, space="PSUM") as psA,
            ):
                XT = [xtp.tile([P, NTOK], F32R, tag=f"xt{cb}", name=f"xt{cb}") for cb in range(NCB)]
                wq_sb = [wp.tile([P, CLOC], F32R, tag=f"wq{cb}", name=f"wq{cb}") for cb in range(NCB)]
                wk_sb = [wp.tile([P, CLOC], F32R, tag=f"wk{cb}", name=f"wk{cb}") for cb in range(NCB)]
                wv_sb = [wp.tile([P, CLOC], F32R, tag=f"wv{cb}", name=f"wv{cb}") for cb in range(NCB)]
                for cb in range(NCB):
                    sl = slice(cb * P, (cb + 1) * P)
                    nc.sync.dma_start(wq_sb[cb][:], Wq[sl, :].bitcast(F32R))
                    nc.sync.dma_start(wk_sb[cb][:], Wk[sl, :].bitcast(F32R))
                    nc.sync.dma_start(wv_sb[cb][:], Wv[sl, :].bitcast(F32R))

                # transpose X: [t, c] -> XT[c][:, t]
                for tb in range(NTB):
                    x_t = xinp.tile([P, DIN], F32, tag="x")
                    nc.sync.dma_start(x_t[:], Xl[tb * P:(tb + 1) * P, :])
                    for cb in range(NCB):
                        tp_ps = psA.tile([P, P], F32, tag="tp", bufs=2)
                        nc.tensor.transpose(
                            tp_ps[:], x_t[:, cb * P:(cb + 1) * P], ident_sb[:]
                        )
                        nc.any.tensor_copy(
                            XT[cb][:, tb * P:(tb + 1) * P], tp_ps[:].bitcast(F32R)
                        )

                # qT / kT (pair-stacked, [128, 2048] each)
                for p in range(2):
                    csl = slice(p * P, (p + 1) * P)
                    for qc in range(NQC):
                        tsl = slice(qc * 512, (qc + 1) * 512)
                        q_ps = psA.tile([P, 512], F32, tag="qkv", bufs=3)
                        for cb in range(NCB):
                            nc.tensor.matmul(
                                q_ps[:], wq_sb[cb][:, csl], XT[cb][:, tsl],
                                start=(cb == 0), stop=(cb == NCB - 1),
                            )
                        nc.any.tensor_copy(qT[p][:, tsl], q_ps[:].bitcast(F32R))
                        k_ps = psA.tile([P, 512], F32, tag="qkv", bufs=3)
                        for cb in range(NCB):
                            nc.tensor.matmul(
                                k_ps[:], wk_sb[cb][:, csl], XT[cb][:, tsl],
                                start=(cb == 0), stop=(cb == NCB - 1),
                            )
                        nc.any.tensor_copy(kT[p][:, tsl], k_ps[:].bitcast(F32R))

                # v tiles [128, 192] = [vA | ones | pad | ones | pad | vB]
                for tb in range(NTB):
                    v_ps = psA.tile([P, CLOC], F32, tag="v", bufs=2)
                    for cb in range(NCB):
                        nc.tensor.matmul(
                            v_ps[:], XT[cb][:, tb * P:(tb + 1) * P], wv_sb[cb][:],
                            start=(cb == 0), stop=(cb == NCB - 1),
                        )
                    for p in range(2):
                        hA, hB = 2 * p, 2 * p + 1
                        t = vt[p][tb]
                        nc.any.tensor_copy(
                            t[:, 0:64], v_ps[:, hA * 64:(hA + 1) * 64].bitcast(F32R)
                        )
                        nc.any.tensor_copy(
                            t[:, 128:192], v_ps[:, hB * 64:(hB + 1) * 64].bitcast(F32R)
                        )
                        nc.any.tensor_copy(t[:, 64:65], ones_sb[:, 0:1])
                        nc.any.tensor_copy(t[:, 96:97], ones_sb[:, 0:1])

            # ---------------- phase B: attention ----------------
            with (
                tc.tile_pool(name="att", bufs=1) as attp,
                tc.tile_pool(name="psB", bufs=1, space="PSUM") as psB,
            ):
                for p in range(2):
                    for qc in range(NQC):
                        qsl = slice(qc * 512, (qc + 1) * 512)
                        nkb = 4 * qc + 4
                        ctx1 = psB.tile([P, 512], F32, tag="ctx1")
                        ctx2 = psB.tile([P, 512], F32, tag="ctx2")
                        for kb in range(nkb):
                            ksl = slice(kb * P, (kb + 1) * P)
                            s_ps = psB.tile([P, 1024], F32, tag="s", bufs=2)
                            nc.tensor.matmul(
                                s_ps[:, 0:512], kT[p][0:64, ksl], qT[p][0:64, qsl],
                                start=True, stop=True, tile_position=(0, 0),
                            )
                            nc.tensor.matmul(
                                s_ps[:, 512:1024], kT[p][64:128, ksl],
                                qT[p][64:128, qsl],
                                start=True, stop=True, tile_position=(64, 0),
                            )
                            oi = kb - 4 * qc
                            if oi >= 0:
                                nc.vector.tensor_add(
                                    s_ps[:, 0:512], s_ps[:, 0:512], mask_sb[:, oi]
                                )
                                nc.vector.tensor_add(
                                    s_ps[:, 512:1024], s_ps[:, 512:1024],
                                    mask_sb[:, oi],
                                )
                            expT = attp.tile([P, 1024], F32R, tag="exp", bufs=3)
                            nc.scalar.activation(expT[:], s_ps[:], EXP, scale=0.125)
                            st, sp = kb == 0, kb == nkb - 1
                            nc.tensor.matmul(
                                ctx1[:], vt[p][kb][:, 0:128], expT[:, 0:512],
                                start=st, stop=sp,
                            )
                            nc.tensor.matmul(
                                ctx2[:], vt[p][kb][:, 64:192], expT[:, 512:1024],
                                start=st, stop=sp,
                            )
                        # normalize: denA at ctx1 row 64, denB at ctx2 row 32
                        rec = attp.tile([P, 512], F32, tag="rec", bufs=2)
                        nc.vector.reciprocal(rec[64:65, :], ctx1[64:65, :])
                        nc.vector.reciprocal(rec[32:33, :], ctx2[32:33, :])
                        rec_r = attp.tile([P, 512], F32R, tag="rec_r", bufs=2)
                        nc.vector.tensor_copy(rec_r[64:65, :], rec[64:65, :])
                        nc.vector.tensor_copy(rec_r[32:33, :], rec[32:33, :])
                        bc_ps = psB.tile([P, 1024], F32, tag="bc")
                        nc.tensor.matmul(
                            bc_ps[:, 0:512], ones_sb[64:65, :], rec_r[64:65, :],
                            start=True, stop=True, tile_position=(64, 0),
                        )
                        nc.tensor.matmul(
                            bc_ps[:, 512:1024], ones_sb[32:33, :], rec_r[32:33, :],
                            start=True, stop=True, tile_position=(32, 0),
                        )
                        bc_sb = attp.tile([P, 1024], F32, tag="bc_sb", bufs=2)
                        nc.any.tensor_copy(bc_sb[:], bc_ps[:])
                        nc.vector.tensor_mul(
                            ctxn[p][0:64, qsl], ctx1[0:64, :], bc_sb[0:64, 0:512]
                        )
                        nc.vector.tensor_mul(
                            ctxn[p][64:128, qsl], ctx2[64:128, :],
                            bc_sb[64:128, 512:1024],
                        )

            # ---------------- phase C: output projection ----------------
            with (
                tc.tile_pool(name="osb", bufs=4) as osbp,
                tc.tile_pool(name="psC", bufs=1, space="PSUM") as psC,
            ):
                for tb in range(NTB):
                    tsl = slice(tb * P, (tb + 1) * P)
                    for jc in range(2):
                        jsl = slice(jc * 512, (jc + 1) * 512)
                        o_ps = psC.tile([P, 512], F32, tag="o", bufs=3)
                        for p in range(2):
                            nc.tensor.matmul(
                                o_ps[:], ctxn[p][:, tsl], wo_sb[p][:, jsl],
                                start=(p == 0), stop=(p == 1),
                            )
                        o_sb = osbp.tile([P, 512], F32, tag="o_sb")
                        nc.any.tensor_copy(o_sb[:], o_ps[:])
                        nc.sync.dma_start(out[tsl, jsl], o_sb[:])

    nc.compile()
    return nc


def _get_nc():
    if "nc" not in _CACHE:
        _CACHE["nc"] = _build()
    return _CACHE["nc"]


def _make_masks():
    m = np.zeros((4, 128, 512), dtype=np.float32)
    kp = np.arange(128)[:, None]
    qf = np.arange(512)[None, :]
    for o in range(4):
        m[o] = np.where(kp + 128 * o <= qf, 0.0, NEG)
    return m


def kernel(X, W_q, W_k, W_v, W_o, b_o):
    from concourse.bass_utils import run_bass_kernel_spmd

    X = np.ascontiguousarray(np.asarray(X, dtype=np.float32))
    W_q = np.ascontiguousarray(np.asarray(W_q, dtype=np.float32))
    W_k = np.ascontiguousarray(np.asarray(W_k, dtype=np.float32))
    W_v = np.ascontiguousarray(np.asarray(W_v, dtype=np.float32))
    W_o = np.ascontiguousarray(np.asarray(W_o, dtype=np.float32))
    b_o = np.asarray(b_o, dtype=np.float32)

    nc = _get_nc()
    masks = _make_masks()
    onesrow = np.ones((1, 128), dtype=np.float32)
    ident = np.eye(128, dtype=np.float32)

    in_maps = []
    for c in range(NCORES):
        b = c // 4
        hb = c % 4
        cs = slice(hb * CLOC, (hb + 1) * CLOC)
        in_maps.append({
            "Xl": X[b],
            "Wq": np.ascontiguousarray(W_q[:, cs]),
            "Wk": np.ascontiguousarray(W_k[:, cs]),
            "Wv": np.ascontiguousarray(W_v[:, cs]),
            "Wo": np.ascontiguousarray(W_o[cs, :]),
            "masks": masks,
            "onesrow": onesrow,
            "ident": ident,
        })

    _CACHE["in_maps"] = in_maps
    global _last_in_maps
    _last_in_maps = in_maps
    res = run_bass_kernel_spmd(nc, in_maps, list(range(NCORES)))
    out = np.empty((B, NTOK, DIN), dtype=np.float32)
    for b in range(B):
        acc = res.results[4 * b]["out"].astype(np.float32)
        for hb in range(1, 4):
            acc = acc + res.results[4 * b + hb]["out"]
        out[b] = acc + b_o[None, :]
    return out
